# revision 7
# baseline (speedup 1.0000x reference)
"""Trainium2 Bass kernel for CNN backbone + top-2 MoE head (B=4096).

Data-parallel over 8 NeuronCores (512 images each). All conv matmuls run in
single-pass fp16 (rel_err ~5e-4 vs the 2e-2 gate; verified by host sim):
  conv1: host-built quarter im2col (K=108: 4 row-quarters x 27 taps),
         M=128 (4 quarters x 32 out-ch), one matmul per 2 images.
  conv2: row bands (K=128: 4 pooled rows x 32 ch), M=128 (2 out-rows x
         64 out-ch); 3 dx-shift matmuls per band.
  conv3: 2-tap pairing: an x-shifted copy of the 64-ch input in partitions
         64..127 lets one K=128 matmul cover taps (dy,0)+(dy,1); tap
         (dy,2) runs at K=64. 6 matmuls per image group instead of 9.
BN is folded into conv weights/biases host-side. PSUM eviction work is
split across engines (only DVE/Act can read PSUM): Act evicts most conv1
slices and all of conv2/conv3 with fused bias+relu; DVE takes the rest of
conv1 via tensor_scalar(add,max), plus conv2/conv3 pooling; GpSimd pools
conv1 in fp16 and issues the band-assembly DMAs (cheap Pool-queue issue).
Gate runs in exact fp32; experts in fp16.
"""
import os
import numpy as np
import ml_dtypes

import concourse.bass as bass
import concourse.mybir as mybir
import concourse.tile as tile
from concourse import bacc
from concourse.bass_utils import run_bass_kernel_spmd
from concourse.masks import make_identity

F32 = mybir.dt.float32
F16 = mybir.dt.float16

N_CORES = 8
B_FULL = 4096
BC = B_FULL // N_CORES      # 512 images per core
MEGA = 32                   # images per pipeline chunk
NMEGA = BC // MEGA
BN_EPS = 1e-5

f16 = np.float16

_cache = {}
last_result = None


# ---------------------------------------------------------------- host prep

def _fold_bn(w, b, g, beta, mean, var):
    inv = g / np.sqrt(var + BN_EPS)
    wf = w * inv[:, None, None, None]
    bf_ = (b - mean) * inv + beta
    return wf.astype(np.float32), bf_.astype(np.float32)


def _arr1(w):
    """conv1 lhsT [108, 128]: p=(q*27 + c*9 + dy*3 + dx), m=(q*32 + o)."""
    out = np.zeros((108, 128), np.float32)
    for q in range(4):
        for c in range(3):
            for dy in range(3):
                for dx in range(3):
                    out[q * 27 + c * 9 + dy * 3 + dx, q * 32:(q + 1) * 32] = \
                        w[:, c, dy, dx]
    return out


def _arr2(w, dxi):
    """conv2 lhsT [128, 128]: p=(rr*32 + c), m=(yloc*64 + o)."""
    out = np.zeros((128, 128), np.float32)
    for rr in range(4):
        for c in range(32):
            for yloc in range(2):
                dy = rr - yloc
                if 0 <= dy <= 2:
                    out[rr * 32 + c, yloc * 64:(yloc + 1) * 64] = w[:, c, dy, dxi]
    return out


def _arr3(w, dy, dx):
    """conv3 per-tap lhsT [64, 128]: p=c, m=o."""
    return np.ascontiguousarray(w[:, :, dy, dx].T)  # [c, o]


def _build_xq(x):
    """Host-side conv1 quarter im2col: [108, B, 256] fp16.

    Partition p = q*27 + c*9 + dy*3 + dx; column n = ry*32 + xx within
    quarter q (out pixel y = q*8+ry); value = x[b, c, y+dy-1, xx+dx-1],
    zero-padded.
    """
    n = x.shape[0]
    xpad = np.zeros((n, 3, 34, 34), np.float32)
    xpad[:, :, 1:33, 1:33] = x.astype(f16).astype(np.float32)
    xq = np.zeros((108, n, 256), np.float32)
    for q in range(4):
        for c in range(3):
            for dy in range(3):
                for dx in range(3):
                    p = q * 27 + c * 9 + dy * 3 + dx
                    sl = xpad[:, c, q * 8 + dy:q * 8 + dy + 8, dx:dx + 32]
                    xq[p] = sl.reshape(n, 256)
    return xq.astype(f16)


def _prep_weights(inp):
    w1f, b1f = _fold_bn(inp['conv1_w'], inp['conv1_b'], inp['bn1_g'],
                        inp['bn1_b'], inp['bn1_m'], inp['bn1_v'])
    w2f, b2f = _fold_bn(inp['conv2_w'], inp['conv2_b'], inp['bn2_g'],
                        inp['bn2_b'], inp['bn2_m'], inp['bn2_v'])
    w3f, b3f = _fold_bn(inp['conv3_w'], inp['conv3_b'], inp['bn3_g'],
                        inp['bn3_b'], inp['bn3_m'], inp['bn3_v'])

    d = {}
    d['w1p'] = _arr1(w1f).astype(f16)
    d['b1v'] = np.tile(b1f, 4).reshape(128, 1)
    d['w2p'] = np.stack([_arr2(w2f, i) for i in range(3)]).astype(f16)
    d['b2v'] = np.tile(b2f, 2).reshape(128, 1)
    # conv3: paired taps (dy,0)+(dy,1) at K=128; tap (dy,2) at K=64
    d['w3pair'] = np.stack([
        np.concatenate([_arr3(w3f, dy, 0), _arr3(w3f, dy, 1)], 0)
        for dy in range(3)]).astype(f16)              # [3, 128, 128]
    d['w3last'] = np.stack([_arr3(w3f, dy, 2)
                            for dy in range(3)]).astype(f16)  # [3, 64, 128]
    d['b3v'] = b3f.reshape(128, 1)
    # gate / experts (fold the 1/16 avgpool into gate_w and w1)
    d['gw'] = (inp['gate_w'] / 16.0).astype(np.float32)        # [128, 8]
    d['gb'] = inp['gate_b'].reshape(1, 8).astype(np.float32)
    d['w1e'] = np.ascontiguousarray(
        (inp['w1'] / 16.0).transpose(1, 0, 2)).reshape(128, 512).astype(f16)
    d['b1row'] = inp['b1'].reshape(1, 512).astype(f16)
    d['w2e'] = np.ascontiguousarray(
        inp['w2'].transpose(1, 0, 2)).astype(f16)              # [64, 8, 10]
    d['b2e'] = inp['b2'].astype(f16)                           # [8, 10]
    return d


# ---------------------------------------------------------------- device IR

def _build_nc():
    nc = bacc.Bacc("TRN2", target_bir_lowering=False, debug=False,
                   enable_asserts=True, num_devices=N_CORES)

    xq_d = nc.dram_tensor("xq", [108, BC, 256], F16,
                          kind="ExternalInput").ap()
    wd = {}
    for name, shape, dt in [
            ('w1p', [108, 128], F16), ('b1v', [128, 1], F32),
            ('w2p', [3, 128, 128], F16), ('b2v', [128, 1], F32),
            ('w3pair', [3, 128, 128], F16), ('w3last', [3, 64, 128], F16),
            ('b3v', [128, 1], F32),
            ('gw', [128, 8], F32), ('gb', [1, 8], F32),
            ('w1e', [128, 512], F16), ('b1row', [1, 512], F16),
            ('w2e', [64, 8, 10], F16), ('b2e', [8, 10], F16)]:
        wd[name] = nc.dram_tensor(name, shape, dt, kind="ExternalInput").ap()
    out_d = nc.dram_tensor("out", [BC, 10], F32, kind="ExternalOutput").ap()

    Relu = mybir.ActivationFunctionType.Relu
    Exp = mybir.ActivationFunctionType.Exp
    Add = mybir.AluOpType.add
    Max = mybir.AluOpType.max

    with tile.TileContext(nc) as tc:
        with tc.tile_pool(name="persist", bufs=1) as pp, \
             tc.tile_pool(name="xqp", bufs=2) as xqp, \
             tc.tile_pool(name="work", bufs=3) as wp, \
             tc.tile_pool(name="ps1p", bufs=3, space="PSUM") as ps1p, \
             tc.tile_pool(name="ps", bufs=2, space="PSUM") as psp:

            # --- persistent SBUF tensors
            C1q = pp.tile([128, 4, MEGA, 16], F16)     # (q,ch), r, img, x
            bands = pp.tile([128, 8, MEGA, 18], F16)   # (rr,ch), b2, img, x+halo
            c2 = pp.tile([128, 8, MEGA, 16], F16)      # (yloc,ch), b2, img, x
            mv2 = pp.tile([64, 8, MEGA, 16], F16)
            rm2 = pp.tile([64, 8, MEGA, 16], F16)
            xp3 = pp.tile([128, MEGA, 10, 10], F16)    # ch | ch-xshift, img, r, x
            featT = pp.tile([128, BC], F32)
            featT16 = pp.tile([128, BC], F16)
            ident16 = pp.tile([128, 128], F16)
            ones16 = pp.tile([1, 128], F16)
            ones32 = pp.tile([1, 128], F32)

            nc.vector.memset(bands[:], 0.0)
            nc.vector.memset(xp3[:], 0.0)
            make_identity(nc, ident16[:])
            nc.vector.memset(ones16[:], 1.0)
            nc.vector.memset(ones32[:], 1.0)

            # --- weights to SBUF
            ws = {}
            for name, src in wd.items():
                v = src
                if name in ('w2p', 'w3pair', 'w3last'):
                    v = src.rearrange("t p m -> p t m")
                t = pp.tile(list(v.shape), src.dtype, name="ws_" + name)
                nc.sync.dma_start(out=t[:], in_=v)
                ws[name] = t

            def emit_conv1_s(xq1, s):
                sl = slice(s * 2, (s + 1) * 2)
                ps1 = ps1p.tile([128, 2, 8, 32], F32, tag="psA")
                nc.tensor.matmul(ps1[:], ws['w1p'][:], xq1[:, sl, :],
                                 start=True, stop=True)
                c1o = wp.tile([128, 2, 8, 32], F16, tag="c1o")
                if s % 4 == 3:   # DVE evicts a quarter of the slices
                    nc.vector.tensor_scalar(c1o[:], ps1[:], ws['b1v'][:],
                                            0.0, op0=Add, op1=Max)
                else:            # Act evicts the rest (fused bias+relu)
                    nc.scalar.activation(c1o[:], ps1[:], Relu,
                                         bias=ws['b1v'][:], scale=1.0)
                rm16 = wp.tile([128, 2, 4, 32], F16, tag="rm16")
                nc.vector.tensor_max(rm16[:], c1o[:, :, 0::2, :],
                                     c1o[:, :, 1::2, :])
                nc.vector.tensor_max(
                    C1q[:, :, sl, :].rearrange("p r g x -> p g r x"),
                    rm16[:, :, :, 0::2], rm16[:, :, :, 1::2])

            def emit_conv3_group(m, s3):
                g3 = slice(s3 * 8, (s3 + 1) * 8)
                ps3 = psp.tile([128, 8, 8, 8], F32, tag="psC")
                for dy in range(3):
                    nc.tensor.matmul(
                        ps3[:], ws['w3pair'][:, dy, :],
                        xp3[0:128, g3, dy:dy + 8, 0:8],
                        start=(dy == 0), stop=False)
                for dy in range(3):
                    nc.tensor.matmul(
                        ps3[:], ws['w3last'][:, dy, :],
                        xp3[0:64, g3, dy:dy + 8, 2:10],
                        start=False, stop=(dy == 2))
                c3o = wp.tile([128, 8, 8, 8], F16, tag="c3o")
                nc.scalar.activation(c3o[:], ps3[:], Relu,
                                     bias=ws['b3v'][:], scale=1.0)
                rm3 = wp.tile([128, 8, 4, 8], F16, tag="rm3")
                nc.vector.tensor_max(rm3[:], c3o[:, :, 0::2, :],
                                     c3o[:, :, 1::2, :])
                cm3 = wp.tile([128, 8, 4, 4], F16, tag="cm3")
                nc.vector.tensor_max(cm3[:], rm3[:, :, :, 0::2],
                                     rm3[:, :, :, 1::2])
                fsl = slice(m * MEGA + s3 * 8, m * MEGA + s3 * 8 + 8)
                nc.vector.tensor_reduce(
                    featT[:, fsl], cm3[:],
                    axis=mybir.AxisListType.XY, op=mybir.AluOpType.add)

            for mega in range(NMEGA):
                g0 = mega * MEGA
                # ---- conv1 im2col chunk: one contiguous DMA (prefetched)
                xq1 = xqp.tile([108, MEGA, 256], F16, tag="xq1")
                nc.sync.dma_start(out=xq1[:], in_=xq_d[:, g0:g0 + MEGA, :])

                # ---- conv1 (interleaved with conv3 of the previous mega
                # to keep the PE queue fed while evictors drain PSUM)
                for s in range(MEGA // 2):
                    emit_conv1_s(xq1, s)
                    if mega > 0 and s % 4 == 2:
                        emit_conv3_group(mega - 1, s // 4)

                # ---- conv2 band assembly (Pool-engine DMA queue: cheap issue)
                for rr in range(4):
                    for b2 in range(8):
                        yp = 2 * b2 - 1 + rr
                        if not (0 <= yp < 16):
                            continue
                        q, ry = yp // 4, yp % 4
                        nc.gpsimd.dma_start(
                            out=bands[rr * 32:(rr + 1) * 32, b2, :, 1:17],
                            in_=C1q[q * 32:(q + 1) * 32, ry, :, :])

                # ---- conv2 matmuls + fp16 eviction
                for b2 in range(8):
                    ps2 = psp.tile([128, MEGA, 16], F32, tag="psB")
                    for dxi in range(3):
                        nc.tensor.matmul(ps2[:], ws['w2p'][:, dxi, :],
                                         bands[:, b2, :, dxi:dxi + 16],
                                         start=(dxi == 0), stop=(dxi == 2))
                    nc.scalar.activation(c2[:, b2, :, :], ps2[:], Relu,
                                         bias=ws['b2v'][:], scale=1.0)
                # ---- pool2 on DVE (fp16): partition move + max + colmax
                nc.sync.dma_start(out=mv2[:], in_=c2[64:128, :, :, :])
                nc.vector.tensor_max(rm2[:], c2[0:64, :, :, :], mv2[:])
                xp3v = xp3[0:64, :, 1:9, 1:9].rearrange("p g r x -> p r g x")
                nc.vector.tensor_max(xp3v, rm2[:, :, :, 0::2],
                                     rm2[:, :, :, 1::2])
                # x-shifted copy for conv3 tap pairing
                nc.sync.dma_start(out=xp3[64:128, :, :, 0:9],
                                  in_=xp3[0:64, :, :, 1:10])

            # ---- trailing conv3 for the last mega
            for s3 in range(MEGA // 8):
                emit_conv3_group(NMEGA - 1, s3)

            # ---------------- MoE head (gate exact fp32, experts fp16)
            nc.vector.tensor_copy(featT16[:], featT[:])
            for blk in range(BC // 128):
                tsl = slice(blk * 128, (blk + 1) * 128)
                lgp = psp.tile([128, 8], F32, tag="psC")
                nc.tensor.matmul(lgp[:], featT[:, tsl], ws['gw'][:],
                                 start=True, stop=False)
                nc.tensor.matmul(lgp[:], ones32[0:1, :], ws['gb'][:],
                                 start=False, stop=True)
                lg = wp.tile([128, 8], F32, tag="lg")
                nc.scalar.copy(lg[:], lgp[:])
                m1 = wp.tile([128, 1], F32, tag="m1")
                nc.vector.reduce_max(m1[:], lg[:], axis=mybir.AxisListType.X)
                sel1 = wp.tile([128, 8], F32, tag="sel1")
                nc.vector.tensor_scalar(sel1[:], lg[:], m1[:], None,
                                        op0=mybir.AluOpType.is_ge)
                tmp = wp.tile([128, 8], F32, tag="tmp8")
                nc.vector.scalar_tensor_tensor(
                    tmp[:], in0=sel1[:], scalar=-1e30, in1=lg[:],
                    op0=mybir.AluOpType.mult, op1=mybir.AluOpType.add)
                m2 = wp.tile([128, 1], F32, tag="m2")
                nc.vector.reduce_max(m2[:], tmp[:], axis=mybir.AxisListType.X)
                sel = wp.tile([128, 8], F32, tag="sel")
                nc.vector.tensor_scalar(sel[:], lg[:], m2[:], None,
                                        op0=mybir.AluOpType.is_ge)
                negm1 = wp.tile([128, 1], F32, tag="negm1")
                nc.vector.tensor_scalar_mul(negm1[:], m1[:], -1.0)
                ex = wp.tile([128, 8], F32, tag="ex")
                nc.scalar.activation(ex[:], lg[:], Exp, bias=negm1[:], scale=1.0)
                e2 = wp.tile([128, 8], F32, tag="e2")
                nc.vector.tensor_mul(e2[:], ex[:], sel[:])
                ssum = wp.tile([128, 1], F32, tag="ssum")
                nc.vector.reduce_sum(ssum[:], e2[:], axis=mybir.AxisListType.X)
                rcp = wp.tile([128, 1], F32, tag="rcp")
                nc.vector.reciprocal(rcp[:], ssum[:])
                wt = wp.tile([128, 8], F32, tag="wt")
                nc.vector.tensor_scalar(wt[:], e2[:], rcp[:], None,
                                        op0=mybir.AluOpType.mult)
                # wt.T (fp16) via PE transpose
                wt16 = wp.tile([128, 8], F16, tag="wt16")
                nc.vector.tensor_copy(wt16[:], wt[:])
                wtp = psp.tile([8, 128], F16, tag="psB")
                nc.tensor.transpose(wtp[:], wt16[:], ident16[:])
                wtT = wp.tile([8, 128], F16, tag="wtT")
                nc.scalar.copy(wtT[:], wtp[:])

                # experts: one batched mm1, per-expert weight+transpose+mm2
                hep = psp.tile([128, 8, 64], F32, tag="psC")
                nc.tensor.matmul(hep[:], featT16[:, tsl], ws['w1e'][:],
                                 start=True, stop=False)
                nc.tensor.matmul(hep[:], ones16[0:1, :], ws['b1row'][:],
                                 start=False, stop=True)
                he = wp.tile([128, 8, 64], F16, tag="he")
                nc.scalar.activation(he[:], hep[:], Relu, scale=1.0)
                hes = wp.tile([128, 8, 64], F16, tag="hes")
                for e in range(8):
                    nc.vector.tensor_scalar(hes[:, e, :], he[:, e, :],
                                            wt[:, e:e + 1], None,
                                            op0=mybir.AluOpType.mult)
                out_ps = psp.tile([128, 10], F32, tag="psC")
                for e in range(8):
                    hTp = psp.tile([64, 128], F16, tag="psB")
                    nc.tensor.transpose(hTp[:], hes[:, e, :], ident16[:])
                    hT = wp.tile([64, 128], F16, tag="hT")
                    nc.scalar.copy(hT[:], hTp[:])
                    nc.tensor.matmul(out_ps[:], hT[:], ws['w2e'][:, e, :],
                                     start=(e == 0), stop=False)
                nc.tensor.matmul(out_ps[:], wtT[:], ws['b2e'][:],
                                 start=False, stop=True)
                outS = wp.tile([128, 10], F32, tag="outS")
                nc.scalar.copy(outS[:], out_ps[:])
                nc.sync.dma_start(out=out_d[tsl, :], in_=outS[:])

    nc.compile()
    return nc


# ---------------------------------------------------------------- entry

def kernel(**inputs):
    global last_result
    if "nc" not in _cache:
        _cache["nc"] = _build_nc()
    nc = _cache["nc"]

    w = _prep_weights(inputs)
    x = np.asarray(inputs['x'], np.float32)
    xq = _build_xq(x)  # [108, B, 256] fp16

    in_maps = []
    for c in range(N_CORES):
        sl = slice(c * BC, (c + 1) * BC)
        m = {'xq': np.ascontiguousarray(xq[:, sl])}
        for k, v in w.items():
            m[k] = v
        in_maps.append(m)

    trace = bool(int(os.environ.get("KERNEL_TRACE", "0")))
    res = run_bass_kernel_spmd(nc, in_maps, core_ids=list(range(N_CORES)),
                               trace=trace)
    last_result = res
    out = np.concatenate([res.results[c]["out"] for c in range(N_CORES)], 0)
    return out.astype(np.float32)


# revision 9
# speedup vs baseline: 3.4683x; 3.4683x over previous
"""Trainium2 Bass kernel for CNN backbone + top-2 MoE head (B=4096).

Data-parallel over 8 NeuronCores (512 images each). All conv matmuls run in
single-pass fp16 (rel_err ~6e-4 vs the 2e-2 gate; verified by host sim):
  conv1: host-built quarter im2col (K=108: 4 row-quarters x 27 taps),
         M=128 (4 quarters x 32 out-ch), one matmul per 2 images.
  conv2: row bands (K=128: 4 pooled rows x 32 ch), M=128 (2 out-rows x
         64 out-ch); 3 dx-shift matmuls per band with asymmetric PSUM
         windows (no halo columns; edge taps fall on zero padding).
  conv3: 2-tap pairing: an x-shifted copy of the 64-ch input in partitions
         64..127 lets one K=128 matmul cover taps (dy,0)+(dy,1); tap
         (dy,2) runs at K=64. 6 matmuls per image group instead of 9.
BN is folded into conv weights/biases host-side. PSUM eviction is split
across engines (only DVE/Act read PSUM): Act evicts 3/4 of conv1 and all
of conv2/conv3 with fused bias+relu; DVE takes 1/4 of conv1 via
tensor_scalar(add,max) plus all fp16 pooling. Band assembly runs as 16
merged contiguous DMAs per 64-image chunk (8 on SP, 8 on the Pool queue);
the conv3 x-shift copy is a single flat contiguous DMA. Gate is exact
fp32; experts fp16.
"""
import os
import numpy as np
import ml_dtypes

import concourse.bass as bass
import concourse.mybir as mybir
import concourse.tile as tile
from concourse import bacc
from concourse.bass_utils import run_bass_kernel_spmd
from concourse.masks import make_identity

F32 = mybir.dt.float32
F16 = mybir.dt.float16

N_CORES = 8
B_FULL = 4096
BC = B_FULL // N_CORES      # 512 images per core
MEGA = 64                   # images per pipeline chunk
NMEGA = BC // MEGA
BN_EPS = 1e-5

f16 = np.float16

_cache = {}
last_result = None


# ---------------------------------------------------------------- host prep

def _fold_bn(w, b, g, beta, mean, var):
    inv = g / np.sqrt(var + BN_EPS)
    wf = w * inv[:, None, None, None]
    bf_ = (b - mean) * inv + beta
    return wf.astype(np.float32), bf_.astype(np.float32)


def _arr1(w):
    """conv1 lhsT [108, 128]: p=(q*27 + c*9 + dy*3 + dx), m=(q*32 + o)."""
    out = np.zeros((108, 128), np.float32)
    for q in range(4):
        for c in range(3):
            for dy in range(3):
                for dx in range(3):
                    out[q * 27 + c * 9 + dy * 3 + dx, q * 32:(q + 1) * 32] = \
                        w[:, c, dy, dx]
    return out


def _arr2(w, dxi):
    """conv2 lhsT [128, 128]: p=(rr*32 + c), m=(yloc*64 + o)."""
    out = np.zeros((128, 128), np.float32)
    for rr in range(4):
        for c in range(32):
            for yloc in range(2):
                dy = rr - yloc
                if 0 <= dy <= 2:
                    out[rr * 32 + c, yloc * 64:(yloc + 1) * 64] = w[:, c, dy, dxi]
    return out


def _arr3(w, dy, dx):
    """conv3 per-tap lhsT [64, 128]: p=c, m=o."""
    return np.ascontiguousarray(w[:, :, dy, dx].T)  # [c, o]


def _build_xq(x):
    """Host-side conv1 quarter im2col: [108, B, 256] fp16.

    Partition p = q*27 + c*9 + dy*3 + dx; column n = ry*32 + xx within
    quarter q (out pixel y = q*8+ry); value = x[b, c, y+dy-1, xx+dx-1],
    zero-padded.
    """
    n = x.shape[0]
    xpad = np.zeros((n, 3, 34, 34), np.float32)
    xpad[:, :, 1:33, 1:33] = x.astype(f16).astype(np.float32)
    xq = np.zeros((108, n, 256), np.float32)
    for q in range(4):
        for c in range(3):
            for dy in range(3):
                for dx in range(3):
                    p = q * 27 + c * 9 + dy * 3 + dx
                    sl = xpad[:, c, q * 8 + dy:q * 8 + dy + 8, dx:dx + 32]
                    xq[p] = sl.reshape(n, 256)
    return xq.astype(f16)


def _prep_weights(inp):
    w1f, b1f = _fold_bn(inp['conv1_w'], inp['conv1_b'], inp['bn1_g'],
                        inp['bn1_b'], inp['bn1_m'], inp['bn1_v'])
    w2f, b2f = _fold_bn(inp['conv2_w'], inp['conv2_b'], inp['bn2_g'],
                        inp['bn2_b'], inp['bn2_m'], inp['bn2_v'])
    w3f, b3f = _fold_bn(inp['conv3_w'], inp['conv3_b'], inp['bn3_g'],
                        inp['bn3_b'], inp['bn3_m'], inp['bn3_v'])

    d = {}
    d['w1p'] = _arr1(w1f).astype(f16)
    d['b1v'] = np.tile(b1f, 4).reshape(128, 1)
    d['w2p'] = np.stack([_arr2(w2f, i) for i in range(3)]).astype(f16)
    d['b2v'] = np.tile(b2f, 2).reshape(128, 1)
    # conv3: paired taps (dy,0)+(dy,1) at K=128; tap (dy,2) at K=64
    d['w3pair'] = np.stack([
        np.concatenate([_arr3(w3f, dy, 0), _arr3(w3f, dy, 1)], 0)
        for dy in range(3)]).astype(f16)              # [3, 128, 128]
    d['w3last'] = np.stack([_arr3(w3f, dy, 2)
                            for dy in range(3)]).astype(f16)  # [3, 64, 128]
    d['b3v'] = b3f.reshape(128, 1)
    # gate / experts (fold the 1/16 avgpool into gate_w and w1)
    d['gw'] = (inp['gate_w'] / 16.0).astype(np.float32)        # [128, 8]
    d['gb'] = inp['gate_b'].reshape(1, 8).astype(np.float32)
    d['w1e'] = np.ascontiguousarray(
        (inp['w1'] / 16.0).transpose(1, 0, 2)).reshape(128, 512).astype(f16)
    d['b1row'] = inp['b1'].reshape(1, 512).astype(f16)
    d['w2e'] = np.ascontiguousarray(
        inp['w2'].transpose(1, 0, 2)).astype(f16)              # [64, 8, 10]
    d['b2e'] = inp['b2'].astype(f16)                           # [8, 10]
    return d


def _band_dmas():
    """Merged band-assembly DMAs: per (rr, q) move the 1-2 bands whose row
    yp = 2*b2-1+rr lands in quarter q. Returns (rr, q, ry0, b20, cnt)."""
    out = []
    for rr in range(4):
        j0 = (rr + 1) % 2
        for q in range(4):
            bs = []
            for j in (j0, j0 + 2):
                yp = 4 * q + j
                b2 = (yp + 1 - rr) // 2
                if 0 <= b2 < 8 and 2 * b2 - 1 + rr == yp:
                    bs.append((j, b2))
            if not bs:
                continue
            ry0, b20 = bs[0]
            out.append((rr, q, ry0, b20, len(bs)))
    return out


# ---------------------------------------------------------------- device IR

def _build_nc():
    nc = bacc.Bacc("TRN2", target_bir_lowering=False, debug=False,
                   enable_asserts=True, num_devices=N_CORES)

    xq_d = nc.dram_tensor("xq", [108, BC, 256], F16,
                          kind="ExternalInput").ap()
    wd = {}
    for name, shape, dt in [
            ('w1p', [108, 128], F16), ('b1v', [128, 1], F32),
            ('w2p', [3, 128, 128], F16), ('b2v', [128, 1], F32),
            ('w3pair', [3, 128, 128], F16), ('w3last', [3, 64, 128], F16),
            ('b3v', [128, 1], F32),
            ('gw', [128, 8], F32), ('gb', [1, 8], F32),
            ('w1e', [128, 512], F16), ('b1row', [1, 512], F16),
            ('w2e', [64, 8, 10], F16), ('b2e', [8, 10], F16)]:
        wd[name] = nc.dram_tensor(name, shape, dt, kind="ExternalInput").ap()
    out_d = nc.dram_tensor("out", [BC, 10], F32, kind="ExternalOutput").ap()

    Relu = mybir.ActivationFunctionType.Relu
    Exp = mybir.ActivationFunctionType.Exp
    Add = mybir.AluOpType.add
    Max = mybir.AluOpType.max

    with tile.TileContext(nc) as tc:
        with tc.tile_pool(name="persist", bufs=1) as pp, \
             tc.tile_pool(name="xqp", bufs=2) as xqp, \
             tc.tile_pool(name="work", bufs=3) as wp, \
             tc.tile_pool(name="ps1p", bufs=3, space="PSUM") as ps1p, \
             tc.tile_pool(name="ps", bufs=2, space="PSUM") as psp:

            # --- persistent SBUF tensors
            C1q = pp.tile([128, 4, MEGA, 16], F16)     # (q,ch), ry, img, x
            bands = pp.tile([128, 8, MEGA, 16], F16)   # (rr,ch), b2, img, x
            c2 = pp.tile([128, 8, MEGA, 16], F16)      # (yloc,ch), b2, img, x
            mv2 = pp.tile([64, 8, MEGA, 16], F16)
            rm2 = pp.tile([64, 8, MEGA, 16], F16)
            xp3 = pp.tile([128, MEGA, 10, 10], F16)    # ch | ch-xshift, img, r, x
            featT = pp.tile([128, BC], F32)
            featT16 = pp.tile([128, BC], F16)
            ident16 = pp.tile([128, 128], F16)
            ones16 = pp.tile([1, 128], F16)
            ones32 = pp.tile([1, 128], F32)

            nc.vector.memset(xp3[:], 0.0)
            # bands slots for pad rows (rr=0,b2=0) and (rr=3,b2=7) are never
            # DMA'd; zero once so the conv2 matmuls read true zero padding
            nc.vector.memset(bands[:], 0.0)
            make_identity(nc, ident16[:])
            nc.vector.memset(ones16[:], 1.0)
            nc.vector.memset(ones32[:], 1.0)

            # --- weights to SBUF
            ws = {}
            for name, src in wd.items():
                v = src
                if name in ('w2p', 'w3pair', 'w3last'):
                    v = src.rearrange("t p m -> p t m")
                t = pp.tile(list(v.shape), src.dtype, name="ws_" + name)
                nc.sync.dma_start(out=t[:], in_=v)
                ws[name] = t

            def emit_conv1_s(xq1, s):
                sl = slice(s * 2, (s + 1) * 2)
                ps1 = ps1p.tile([128, 2, 8, 32], F32, tag="psA")
                nc.tensor.matmul(ps1[:], ws['w1p'][:], xq1[:, sl, :],
                                 start=True, stop=True)
                c1o = wp.tile([128, 2, 8, 32], F16, tag="c1o")
                if s % 4 == 3:   # DVE evicts a quarter of the slices
                    nc.vector.tensor_scalar(c1o[:], ps1[:], ws['b1v'][:],
                                            0.0, op0=Add, op1=Max)
                else:            # Act evicts the rest (fused bias+relu)
                    nc.scalar.activation(c1o[:], ps1[:], Relu,
                                         bias=ws['b1v'][:], scale=1.0)
                rm16 = wp.tile([128, 2, 4, 32], F16, tag="rm16")
                nc.vector.tensor_max(rm16[:], c1o[:, :, 0::2, :],
                                     c1o[:, :, 1::2, :])
                nc.vector.tensor_max(
                    C1q[:, :, sl, :].rearrange("p r g x -> p g r x"),
                    rm16[:, :, :, 0::2], rm16[:, :, :, 1::2])

            def emit_conv3_group(m, s3):
                g3 = slice(s3 * 8, (s3 + 1) * 8)
                ps3 = psp.tile([128, 8, 8, 8], F32, tag="psC")
                for dy in range(3):
                    nc.tensor.matmul(
                        ps3[:], ws['w3pair'][:, dy, :],
                        xp3[0:128, g3, dy:dy + 8, 0:8],
                        start=(dy == 0), stop=False)
                for dy in range(3):
                    nc.tensor.matmul(
                        ps3[:], ws['w3last'][:, dy, :],
                        xp3[0:64, g3, dy:dy + 8, 2:10],
                        start=False, stop=(dy == 2))
                c3o = wp.tile([128, 8, 8, 8], F16, tag="c3o")
                nc.scalar.activation(c3o[:], ps3[:], Relu,
                                     bias=ws['b3v'][:], scale=1.0)
                rm3 = wp.tile([128, 8, 4, 8], F16, tag="rm3")
                nc.vector.tensor_max(rm3[:], c3o[:, :, 0::2, :],
                                     c3o[:, :, 1::2, :])
                cm3 = wp.tile([128, 8, 4, 4], F16, tag="cm3")
                nc.vector.tensor_max(cm3[:], rm3[:, :, :, 0::2],
                                     rm3[:, :, :, 1::2])
                fsl = slice(m * MEGA + s3 * 8, m * MEGA + s3 * 8 + 8)
                nc.vector.tensor_reduce(
                    featT[:, fsl], cm3[:],
                    axis=mybir.AxisListType.XY, op=mybir.AluOpType.add)

            band_plan = _band_dmas()

            for mega in range(NMEGA):
                g0 = mega * MEGA
                # ---- conv1 im2col chunk: one contiguous DMA (prefetched)
                xq1 = xqp.tile([108, MEGA, 256], F16, tag="xq1")
                nc.sync.dma_start(out=xq1[:], in_=xq_d[:, g0:g0 + MEGA, :])

                # ---- conv1 (interleaved with conv3 of the previous mega
                # to keep the PE queue fed while evictors drain PSUM)
                for s in range(MEGA // 2):
                    emit_conv1_s(xq1, s)
                    if mega > 0 and s % 4 == 2:
                        emit_conv3_group(mega - 1, s // 4)

                # ---- conv2 band assembly: 16 merged contiguous DMAs
                for rr, q, ry0, b20, cnt in band_plan:
                    dst = bands[rr * 32:(rr + 1) * 32, b20:b20 + cnt, :, :]
                    src = C1q[q * 32:(q + 1) * 32, ry0:ry0 + 2 * cnt - 1:2, :, :]
                    eng = nc.sync if rr < 2 else nc.gpsimd
                    eng.dma_start(out=dst, in_=src)

                # ---- conv2 matmuls (asymmetric dx windows) + fp16 eviction
                for b2 in range(8):
                    for h in range(2):
                        hsl = slice(h * 32, (h + 1) * 32)
                        ps2 = psp.tile([128, 32, 16], F32, tag="psB")
                        nc.tensor.matmul(ps2[:], ws['w2p'][:, 1, :],
                                         bands[:, b2, hsl, :],
                                         start=True, stop=False)
                        nc.tensor.matmul(ps2[:, :, 1:16], ws['w2p'][:, 0, :],
                                         bands[:, b2, hsl, 0:15],
                                         start=False, stop=False,
                                         skip_group_check=True)
                        nc.tensor.matmul(ps2[:, :, 0:15], ws['w2p'][:, 2, :],
                                         bands[:, b2, hsl, 1:16],
                                         start=False, stop=True,
                                         skip_group_check=True)
                        nc.scalar.activation(c2[:, b2, hsl, :], ps2[:], Relu,
                                             bias=ws['b2v'][:], scale=1.0)
                # ---- pool2 on DVE (fp16): partition move + max + colmax
                nc.sync.dma_start(out=mv2[:], in_=c2[64:128, :, :, :])
                nc.vector.tensor_max(rm2[:], c2[0:64, :, :, :], mv2[:])
                xp3v = xp3[0:64, :, 1:9, 1:9].rearrange("p g r x -> p r g x")
                nc.vector.tensor_max(xp3v, rm2[:, :, :, 0::2],
                                     rm2[:, :, :, 1::2])
                # x-shifted copy for conv3 tap pairing (flat contiguous DMA)
                lo_flat = xp3[0:64].rearrange("p a b c -> p (a b c)")
                hi_flat = xp3[64:128].rearrange("p a b c -> p (a b c)")
                nc.sync.dma_start(out=hi_flat[:, 0:MEGA * 100 - 1],
                                  in_=lo_flat[:, 1:MEGA * 100])

            # ---- trailing conv3 for the last mega
            for s3 in range(MEGA // 8):
                emit_conv3_group(NMEGA - 1, s3)

            # ---------------- MoE head (gate exact fp32, experts fp16)
            nc.vector.tensor_copy(featT16[:], featT[:])
            for blk in range(BC // 128):
                tsl = slice(blk * 128, (blk + 1) * 128)
                lgp = psp.tile([128, 8], F32, tag="psC")
                nc.tensor.matmul(lgp[:], featT[:, tsl], ws['gw'][:],
                                 start=True, stop=False)
                nc.tensor.matmul(lgp[:], ones32[0:1, :], ws['gb'][:],
                                 start=False, stop=True)
                lg = wp.tile([128, 8], F32, tag="lg")
                nc.scalar.copy(lg[:], lgp[:])
                m1 = wp.tile([128, 1], F32, tag="m1")
                nc.vector.reduce_max(m1[:], lg[:], axis=mybir.AxisListType.X)
                sel1 = wp.tile([128, 8], F32, tag="sel1")
                nc.vector.tensor_scalar(sel1[:], lg[:], m1[:], None,
                                        op0=mybir.AluOpType.is_ge)
                tmp = wp.tile([128, 8], F32, tag="tmp8")
                nc.vector.scalar_tensor_tensor(
                    tmp[:], in0=sel1[:], scalar=-1e30, in1=lg[:],
                    op0=mybir.AluOpType.mult, op1=mybir.AluOpType.add)
                m2 = wp.tile([128, 1], F32, tag="m2")
                nc.vector.reduce_max(m2[:], tmp[:], axis=mybir.AxisListType.X)
                sel = wp.tile([128, 8], F32, tag="sel")
                nc.vector.tensor_scalar(sel[:], lg[:], m2[:], None,
                                        op0=mybir.AluOpType.is_ge)
                negm1 = wp.tile([128, 1], F32, tag="negm1")
                nc.vector.tensor_scalar_mul(negm1[:], m1[:], -1.0)
                ex = wp.tile([128, 8], F32, tag="ex")
                nc.scalar.activation(ex[:], lg[:], Exp, bias=negm1[:], scale=1.0)
                e2 = wp.tile([128, 8], F32, tag="e2")
                nc.vector.tensor_mul(e2[:], ex[:], sel[:])
                ssum = wp.tile([128, 1], F32, tag="ssum")
                nc.vector.reduce_sum(ssum[:], e2[:], axis=mybir.AxisListType.X)
                rcp = wp.tile([128, 1], F32, tag="rcp")
                nc.vector.reciprocal(rcp[:], ssum[:])
                wt = wp.tile([128, 8], F32, tag="wt")
                nc.vector.tensor_scalar(wt[:], e2[:], rcp[:], None,
                                        op0=mybir.AluOpType.mult)
                # wt.T (fp16) via PE transpose
                wt16 = wp.tile([128, 8], F16, tag="wt16")
                nc.vector.tensor_copy(wt16[:], wt[:])
                wtp = psp.tile([8, 128], F16, tag="psB")
                nc.tensor.transpose(wtp[:], wt16[:], ident16[:])
                wtT = wp.tile([8, 128], F16, tag="wtT")
                nc.scalar.copy(wtT[:], wtp[:])

                # experts: one batched mm1, per-expert weight+transpose+mm2
                hep = psp.tile([128, 8, 64], F32, tag="psC")
                nc.tensor.matmul(hep[:], featT16[:, tsl], ws['w1e'][:],
                                 start=True, stop=False)
                nc.tensor.matmul(hep[:], ones16[0:1, :], ws['b1row'][:],
                                 start=False, stop=True)
                he = wp.tile([128, 8, 64], F16, tag="he")
                nc.scalar.activation(he[:], hep[:], Relu, scale=1.0)
                hes = wp.tile([128, 8, 64], F16, tag="hes")
                for e in range(8):
                    nc.vector.tensor_scalar(hes[:, e, :], he[:, e, :],
                                            wt[:, e:e + 1], None,
                                            op0=mybir.AluOpType.mult)
                out_ps = psp.tile([128, 10], F32, tag="psC")
                for e in range(8):
                    hTp = psp.tile([64, 128], F16, tag="psB")
                    nc.tensor.transpose(hTp[:], hes[:, e, :], ident16[:])
                    hT = wp.tile([64, 128], F16, tag="hT")
                    nc.scalar.copy(hT[:], hTp[:])
                    nc.tensor.matmul(out_ps[:], hT[:], ws['w2e'][:, e, :],
                                     start=(e == 0), stop=False)
                nc.tensor.matmul(out_ps[:], wtT[:], ws['b2e'][:],
                                 start=False, stop=True)
                outS = wp.tile([128, 10], F32, tag="outS")
                nc.scalar.copy(outS[:], out_ps[:])
                nc.sync.dma_start(out=out_d[tsl, :], in_=outS[:])

    nc.compile()
    return nc


# ---------------------------------------------------------------- entry

def kernel(**inputs):
    global last_result
    if "nc" not in _cache:
        _cache["nc"] = _build_nc()
    nc = _cache["nc"]

    w = _prep_weights(inputs)
    x = np.asarray(inputs['x'], np.float32)
    xq = _build_xq(x)  # [108, B, 256] fp16

    in_maps = []
    for c in range(N_CORES):
        sl = slice(c * BC, (c + 1) * BC)
        m = {'xq': np.ascontiguousarray(xq[:, sl])}
        for k, v in w.items():
            m[k] = v
        in_maps.append(m)

    trace = bool(int(os.environ.get("KERNEL_TRACE", "0")))
    res = run_bass_kernel_spmd(nc, in_maps, core_ids=list(range(N_CORES)),
                               trace=trace)
    last_result = res
    out = np.concatenate([res.results[c]["out"] for c in range(N_CORES)], 0)
    return out.astype(np.float32)
